# revision 6
# baseline (speedup 1.0000x reference)
"""Trainium2 Bass kernel for nn_ContinuousEmbedding (histogram binning + distance-
weighted embedding mix).

Math: for each scalar x[b,f], the reference computes bucket index
idx = #{j in 1..63 : x > low[j]} and returns
    out[b,f,:] = sum_k weight[k,:] / (|idx-k|+1)  =  T[idx,:]
where T = S @ weight, S[i,k] = 1/(|i-k|+1) is a fixed 64x64 matrix.

T[idx] telescopes over compare signs s_j = sign(x - low[j]) (s_0 = +1 since
low[0] = -inf):
    T[idx] = sum_j s_j * V[j],  V[0] = (T[0]+T[63])/2, V[j] = (T[j]-T[j-1])/2

Device pipeline, per pair of 1024-token chunks (chunks p and p+32 stacked in
the two 64-partition halves so every engine runs 128 partitions wide):

  bcast:  one bf16 matmul with a [6,128] block-diagonal ones stationary
          broadcasts BOTH chunks: x is split on the host into an exact
          hi+mid+lo bf16 triple (3x8 mantissa bits = f32's 24), the three
          strands ride the contraction dim, PSUM accumulates them back to
          exact f32.  1 cycle/column vs 4 for an fp32 matmul.
  sign:   alternating per pair to balance the two elementwise engines:
            ACT path: sg = Sign(xb + (-low)) in {-1,0,+1}, table V
            DVE path: sg = (xb is_gt low)    in {0,1},     table 2V, and the
                      copy step subtracts c = sum_j V[j] via its bias input
          (DVE's strict is_gt matches the reference tie behavior exactly;
          ACT's sign(0)=0 ties are patched on the host.)
  gather: one [128,128] block-diagonal V stationary computes both chunks'
          distance-weighted rows in 1 cycle/column.
  copy:   PSUM -> SBUF fp16 on the engine the sign step didn't use
          (ACT Identity with bias -c for the DVE path, DVE tensor_copy
          otherwise).
  out:    per group of `group` pairs, two HWDGE DMAs into the chunk-major
          [64, 64, 1024] fp16 device output (contiguous 128 KB per chunk);
          the host reorders and upcasts to f32 at unshard time (2e-2 gate;
          fp16 adds ~5e-4).

The PE is software-pipelined: pair p+1's broadcast matmuls are emitted
before pair p's gather matmuls so the tensor engine never idles waiting on
the sign step.
"""

import os as _os
import sys

import numpy as np

for _p in ("/opt/trn_rl_repo",):
    if _p not in sys.path:
        sys.path.insert(0, _p)

import ml_dtypes  # noqa: E402

import concourse.bass as bass  # noqa: E402,F401
import concourse.mybir as mybir  # noqa: E402
import concourse.tile as tile  # noqa: E402
from concourse import bacc  # noqa: E402
from concourse import bass_utils  # noqa: E402

B, F, K, D = 8192, 64, 64, 64
NCORES = 8
NTOK = (B // NCORES) * F          # 65536 tokens per core
CHUNK = 1024                      # tokens per chunk
NCHUNK = NTOK // CHUNK            # 64 chunks
NPAIR = NCHUNK // 2               # 32 chunk pairs (chunk p with chunk p+32)
HALF = 512                        # max f32 psum columns per matmul

FP16 = mybir.dt.float16
BF16 = mybir.dt.bfloat16
F32 = mybir.dt.float32

CFG = {
    "group": 4,        # pairs per output-DMA group
    "xpieces": 4,      # number of staged x6 input DMAs
    "dve_mod2": 1,     # pairs with p%2 < dve_mod2 use the DVE-sign path
}
for _kv in _os.environ.get("KCFG", "").split(","):
    if "=" in _kv:
        _k, _v = _kv.split("=", 1)
        CFG[_k.strip()] = int(_v) if _v.strip().lstrip("-").isdigit() else _v.strip()


def build_tile_kernel(nc, tc, x6_d, ones6_d, v2a_d, v2b_d, lowneg_d, lowpos_d, negc_d, out_d):
    G = CFG["group"]
    XP = CFG["xpieces"]
    assert NPAIR % G == 0 and NPAIR % XP == 0
    PPP = NPAIR // XP                                     # pairs per x6 piece
    # chunk-major output, iterated (d, chunk, tok) so the SBUF source walks
    # partitions outermost
    out_ap = out_d.ap().rearrange("c d n -> d c n")       # [64, 64, 1024]
    x6_ap = x6_d.ap()                                     # [6, NPAIR*CHUNK]

    with tc.tile_pool(name="cpool", bufs=1) as cpool:
        ones6 = cpool.tile([6, 128], BF16)
        nc.sync.dma_start(out=ones6[:], in_=ones6_d.ap())

        xp = []
        for i in range(XP):
            t = cpool.tile([6, PPP * CHUNK], BF16, tag=f"x6p{i}")
            nc.sync.dma_start(
                out=t[:], in_=x6_ap[:, i * PPP * CHUNK : (i + 1) * PPP * CHUNK]
            )
            xp.append(t)
        v2a = cpool.tile([128, 128], FP16)
        nc.sync.dma_start(out=v2a[:], in_=v2a_d.ap())
        v2b = cpool.tile([128, 128], FP16)
        nc.sync.dma_start(out=v2b[:], in_=v2b_d.ap())
        lowneg = cpool.tile([128, 1], F32)
        nc.sync.dma_start(out=lowneg[:], in_=lowneg_d.ap())
        lowpos = cpool.tile([128, 1], F32)
        nc.sync.dma_start(out=lowpos[:], in_=lowpos_d.ap())
        negc = cpool.tile([128, 1], F32)
        nc.sync.dma_start(out=negc[:], in_=negc_d.ap())

        def bcast(pxpool, p):
            xb2 = pxpool.tile([128, CHUNK], F32, tag="xb2")
            src = xp[p // PPP]
            base = (p % PPP) * CHUNK
            for h in range(2):
                nc.tensor.matmul(
                    out=xb2[:, HALF * h : HALF * (h + 1)],
                    lhsT=ones6[:],
                    rhs=src[:, base + HALF * h : base + HALF * (h + 1)],
                    start=True,
                    stop=True,
                )
            return xb2

        with (
            tc.tile_pool(name="spool", bufs=3) as spool,
            tc.tile_pool(name="opool", bufs=2) as opool,
            tc.tile_pool(name="pxpool", bufs=2, space="PSUM") as pxpool,
            tc.tile_pool(name="popool", bufs=2, space="PSUM") as popool,
        ):
            xb2_next = bcast(pxpool, 0)
            ob2 = None
            for p in range(NPAIR):
                xb2 = xb2_next
                k = p % G
                if k == 0:
                    ob2 = opool.tile([128, G * CHUNK], FP16, tag="ob2")
                dve_path = (p % 2) < CFG["dve_mod2"]

                if p + 1 < NPAIR:
                    xb2_next = bcast(pxpool, p + 1)

                sg = spool.tile([128, CHUNK], FP16, tag="sg")
                if dve_path:
                    nc.vector.tensor_scalar(
                        out=sg[:],
                        in0=xb2[:],
                        scalar1=lowpos[:],
                        scalar2=None,
                        op0=mybir.AluOpType.is_gt,
                    )
                else:
                    nc.scalar.activation(
                        out=sg[:],
                        in_=xb2[:],
                        func=mybir.ActivationFunctionType.Sign,
                        bias=lowneg[:],
                        scale=1.0,
                    )

                ps = popool.tile([128, CHUNK], F32, tag="ps")
                vtab = v2b if dve_path else v2a
                for h in range(2):
                    nc.tensor.matmul(
                        out=ps[:, HALF * h : HALF * (h + 1)],
                        lhsT=vtab[:],
                        rhs=sg[:, HALF * h : HALF * (h + 1)],
                        start=True,
                        stop=True,
                    )

                dst = ob2[:, k * CHUNK : (k + 1) * CHUNK]
                if dve_path:
                    nc.scalar.activation(
                        out=dst,
                        in_=ps[:],
                        func=mybir.ActivationFunctionType.Identity,
                        bias=negc[:],
                        scale=1.0,
                    )
                else:
                    nc.vector.tensor_copy(out=dst, in_=ps[:])

                if k == G - 1:
                    g0 = p + 1 - G
                    nc.sync.dma_start(
                        out=out_ap[:, g0 : g0 + G, :],
                        in_=ob2[0:64, :].rearrange("d (c n) -> d c n", c=G),
                    )
                    nc.sync.dma_start(
                        out=out_ap[:, NPAIR + g0 : NPAIR + g0 + G, :],
                        in_=ob2[64:128, :].rearrange("d (c n) -> d c n", c=G),
                    )


_CACHED_NC = None


def _get_nc():
    global _CACHED_NC
    if _CACHED_NC is None:
        nc = bacc.Bacc("TRN2", target_bir_lowering=False, debug=False)
        x6_d = nc.dram_tensor("x6", [6, NPAIR * CHUNK], BF16, kind="ExternalInput")
        ones6_d = nc.dram_tensor("ones6", [6, 128], BF16, kind="ExternalInput")
        v2a_d = nc.dram_tensor("v2a", [128, 128], FP16, kind="ExternalInput")
        v2b_d = nc.dram_tensor("v2b", [128, 128], FP16, kind="ExternalInput")
        lowneg_d = nc.dram_tensor("lowneg", [128, 1], F32, kind="ExternalInput")
        lowpos_d = nc.dram_tensor("lowpos", [128, 1], F32, kind="ExternalInput")
        negc_d = nc.dram_tensor("negc", [128, 1], F32, kind="ExternalInput")
        out_d = nc.dram_tensor("out", [NCHUNK, D, CHUNK], FP16, kind="ExternalOutput")
        with tile.TileContext(nc) as tc:
            build_tile_kernel(
                nc, tc, x6_d, ones6_d, v2a_d, v2b_d, lowneg_d, lowpos_d, negc_d, out_d
            )
        nc.compile()
        _CACHED_NC = nc
    return _CACHED_NC


def make_host_tables(low, weight):
    """Constant device inputs derived from low/weight (float64 on host)."""
    ar = np.arange(K)
    S = 1.0 / (np.abs(ar[:, None] - ar[None, :]) + 1.0)              # [K, K] f64
    T = S @ np.asarray(weight, np.float64)                           # [K, D]
    V = np.empty_like(T)
    V[0] = (T[0] + T[-1]) / 2
    V[1:] = (T[1:] - T[:-1]) / 2

    v2a = np.zeros((128, 128), np.float16)
    v2a[0:64, 0:64] = V.astype(np.float16)
    v2a[64:128, 64:128] = v2a[0:64, 0:64]
    # DVE path: sg in {0,1}; 2V table, and -c = -sum_j V[j] folded into the
    # copy bias.  Use the fp16-rounded table when deriving c so the bias
    # cancels the table's own quantization at sg=all-ones.
    v2b16 = (2.0 * V).astype(np.float16)
    v2b = np.zeros((128, 128), np.float16)
    v2b[0:64, 0:64] = v2b16
    v2b[64:128, 64:128] = v2b16
    c = np.asarray(v2b16, np.float64).sum(axis=0) / 2                # [D] f64
    negc = np.tile((-c).astype(np.float32).reshape(1, 64), (2, 1)).reshape(128, 1)

    lowf = np.asarray(low, np.float64)
    lowneg = np.tile((-lowf).astype(np.float32), 2).reshape(128, 1)
    lowpos = np.tile(lowf.astype(np.float32), 2).reshape(128, 1)
    ones6 = np.zeros((6, 128), ml_dtypes.bfloat16)
    ones6[0:3, 0:64] = 1
    ones6[3:6, 64:128] = 1
    return {"ones6": ones6, "v2a": v2a, "v2b": v2b, "lowneg": lowneg,
            "lowpos": lowpos, "negc": negc}


def make_x6(xcore):
    """Exact hi+mid+lo bf16 split of a core's [NTOK] f32 tokens, packed as
    [6, NPAIR*CHUNK]: strands 0-2 = chunk p, strands 3-5 = chunk p+32."""
    bf = ml_dtypes.bfloat16
    x = np.ascontiguousarray(xcore, np.float32)
    hi = x.astype(bf)
    r1 = x - hi.astype(np.float32)
    mid = r1.astype(bf)
    lo = (r1 - mid.astype(np.float32)).astype(bf)
    x6 = np.empty((6, NPAIR, CHUNK), bf)
    for s, arr in enumerate((hi, mid, lo)):
        c = arr.reshape(NCHUNK, CHUNK)
        x6[s] = c[:NPAIR]
        x6[s + 3] = c[NPAIR:]
    return np.ascontiguousarray(x6.reshape(6, NPAIR * CHUNK))


def host_correct_ties(out2d, xflat, low, weight):
    """Exact fixup for tokens where x equals a bin edge: the ACT-path Sign
    gives sign(0)=0 there (averaging two table rows) while the reference uses
    strict x > low. Replace those few rows with the exact table row."""
    bins = np.asarray(low, np.float32)[1:]
    ties = np.isin(xflat, bins)
    if not ties.any():
        return out2d
    xt = xflat[ties]
    idx = (xt[:, None] > bins[None, :]).sum(-1)
    ar = np.arange(K)
    S = 1.0 / (np.abs(ar[:, None] - ar[None, :]) + 1.0)
    T = (S @ np.asarray(weight, np.float64)).astype(np.float32)
    out2d[ties] = T[idx]
    return out2d


def make_in_maps(x, low, weight):
    tabs = make_host_tables(low, weight)
    shards = np.asarray(x, np.float32).reshape(NCORES, NTOK)
    return [{"x6": make_x6(shards[i]), **tabs} for i in range(NCORES)]


def unshard(res):
    """[NCHUNK, D, CHUNK] fp16 per core -> [NCORES*NTOK, D] f32 token rows."""
    outs = []
    for i in range(NCORES):
        o = np.asarray(res.results[i]["out"])                 # [64, 64, 1024] fp16
        outs.append(o.transpose(0, 2, 1).reshape(NTOK, D).astype(np.float32))
    return np.concatenate(outs, axis=0)


def run_cores(x, low, weight, trace=False):
    nc = _get_nc()
    in_maps = make_in_maps(x, low, weight)
    res = bass_utils.run_bass_kernel_spmd(
        nc, in_maps, core_ids=list(range(NCORES)), trace=trace
    )
    return unshard(res), res


def kernel(x, low, high, weight):
    x = np.asarray(x, np.float32)
    out, _ = run_cores(x, low, weight)
    out = host_correct_ties(out, x.reshape(-1), low, weight)
    return out.reshape(B, F, D)


# revision 8
# speedup vs baseline: 1.1823x; 1.1823x over previous
"""Trainium2 Bass kernel for nn_ContinuousEmbedding (histogram binning + distance-
weighted embedding mix).

Math: for each scalar x[b,f], the reference computes bucket index
idx = #{j in 1..63 : x > low[j]} and returns
    out[b,f,:] = sum_k weight[k,:] / (|idx-k|+1)  =  T[idx,:]
where T = S @ weight, S[i,k] = 1/(|i-k|+1) is a fixed 64x64 matrix.

T[idx] telescopes over compare signs s_j = sign(x - low[j]) (s_0 = +1 since
low[0] = -inf):
    T[idx] = sum_j s_j * V[j],  V[0] = (T[0]+T[63])/2, V[j] = (T[j]-T[j-1])/2

Device pipeline, per pair of 1024-token chunks (chunks p and p+32 stacked in
the two 64-partition halves so every engine runs 128 partitions wide):

  bcast:  one bf16 matmul with a [6,128] block-diagonal ones stationary
          broadcasts BOTH chunks: x is split on the host into an exact
          hi+mid+lo bf16 triple (3x8 mantissa bits = f32's 24), the three
          strands ride the contraction dim, PSUM accumulates them back to
          exact f32.  1 cycle/column vs 4 for an fp32 matmul.
  sign:   alternating per pair to balance the two elementwise engines:
            ACT path: sg = Sign(xb + (-low)) in {-1,0,+1}, table V
            DVE path: sg = (xb is_gt low)    in {0,1},     table 2V, and the
                      copy step subtracts c = sum_j V[j] via its bias input
          (DVE's strict is_gt matches the reference tie behavior exactly;
          ACT's sign(0)=0 ties are patched on the host.)
  gather: one [128,128] block-diagonal V stationary computes both chunks'
          distance-weighted rows in 1 cycle/column.
  copy:   PSUM -> SBUF fp16 on the engine the sign step didn't use
          (ACT Identity with bias -c for the DVE path, DVE tensor_copy
          otherwise).
  out:    per group of `group` pairs, two HWDGE DMAs into the chunk-major
          [64, 64, 1024] fp16 device output (contiguous 128 KB per chunk);
          the host reorders and upcasts to f32 at unshard time (2e-2 gate;
          fp16 adds ~5e-4).

The PE is software-pipelined: pair p+1's broadcast matmuls are emitted
before pair p's gather matmuls so the tensor engine never idles waiting on
the sign step.
"""

import os as _os
import sys

import numpy as np

for _p in ("/opt/trn_rl_repo",):
    if _p not in sys.path:
        sys.path.insert(0, _p)

import ml_dtypes  # noqa: E402

import concourse.bass as bass  # noqa: E402,F401
import concourse.mybir as mybir  # noqa: E402
import concourse.tile as tile  # noqa: E402
from concourse import bacc  # noqa: E402
from concourse import bass_utils  # noqa: E402

B, F, K, D = 8192, 64, 64, 64
NCORES = 8
NTOK = (B // NCORES) * F          # 65536 tokens per core
CHUNK = 1024                      # tokens per chunk
NCHUNK = NTOK // CHUNK            # 64 chunks
NPAIR = NCHUNK // 2               # 32 chunk pairs (chunk p with chunk p+32)
HALF = 512                        # max f32 psum columns per matmul

FP16 = mybir.dt.float16
BF16 = mybir.dt.bfloat16
F32 = mybir.dt.float32

CFG = {
    "group": 4,        # pairs per output-DMA group
    "xpieces": 4,      # number of staged x6 input DMAs
    "dve_mod2": 1,     # pairs with p%2 < dve_mod2 use the DVE-sign path
    "actboth_mod": 32, # pairs with p%actboth_mod==actboth_mod-1 run sign+copy on ACT
}
for _kv in _os.environ.get("KCFG", "").split(","):
    if "=" in _kv:
        _k, _v = _kv.split("=", 1)
        CFG[_k.strip()] = int(_v) if _v.strip().lstrip("-").isdigit() else _v.strip()


def build_tile_kernel(nc, tc, x6_d, ones6_d, v2a_d, v2b_d, lowneg_d, lowpos_d, negc_d, out_d):
    G = CFG["group"]
    XP = CFG["xpieces"]
    assert NPAIR % G == 0 and NPAIR % XP == 0
    PPP = NPAIR // XP                                     # pairs per x6 piece
    # chunk-major output, iterated (d, chunk, tok) so the SBUF source walks
    # partitions outermost
    out_ap = out_d.ap().rearrange("c d n -> d c n")       # [64, 64, 1024]
    x6_ap = x6_d.ap()                                     # [6, NPAIR*CHUNK]

    with tc.tile_pool(name="cpool", bufs=1) as cpool:
        ones6 = cpool.tile([6, 128], BF16)
        nc.sync.dma_start(out=ones6[:], in_=ones6_d.ap())

        xp = [
            cpool.tile([6, PPP * CHUNK], BF16, tag=f"x6p{i}", name=f"x6p{i}")
            for i in range(XP)
        ]
        nc.gpsimd.dma_start(out=xp[0][:], in_=x6_ap[:, 0 : PPP * CHUNK])
        lowneg = cpool.tile([128, 1], F32)
        nc.gpsimd.dma_start(out=lowneg[:], in_=lowneg_d.ap())
        lowpos = cpool.tile([128, 1], F32)
        nc.gpsimd.dma_start(out=lowpos[:], in_=lowpos_d.ap())
        v2a = cpool.tile([128, 128], FP16)
        nc.gpsimd.dma_start(out=v2a[:], in_=v2a_d.ap())
        v2b = cpool.tile([128, 128], FP16)
        nc.gpsimd.dma_start(out=v2b[:], in_=v2b_d.ap())
        negc = cpool.tile([128, 1], F32)
        nc.gpsimd.dma_start(out=negc[:], in_=negc_d.ap())
        for i in range(1, XP):
            nc.gpsimd.dma_start(
                out=xp[i][:], in_=x6_ap[:, i * PPP * CHUNK : (i + 1) * PPP * CHUNK]
            )

        def bcast(pxpool, p):
            xb2 = pxpool.tile([128, CHUNK], F32, tag="xb2")
            src = xp[p // PPP]
            base = (p % PPP) * CHUNK
            for h in range(2):
                nc.tensor.matmul(
                    out=xb2[:, HALF * h : HALF * (h + 1)],
                    lhsT=ones6[:],
                    rhs=src[:, base + HALF * h : base + HALF * (h + 1)],
                    start=True,
                    stop=True,
                )
            return xb2

        with (
            tc.tile_pool(name="spool", bufs=3) as spool,
            tc.tile_pool(name="opool", bufs=2) as opool,
            tc.tile_pool(name="pxpool", bufs=2, space="PSUM") as pxpool,
            tc.tile_pool(name="popool", bufs=2, space="PSUM") as popool,
        ):
            xb2_next = bcast(pxpool, 0)
            ob2 = None
            for p in range(NPAIR):
                xb2 = xb2_next
                k = p % G
                if k == 0:
                    ob2 = opool.tile([128, G * CHUNK], FP16, tag="ob2")
                act_both = (p % CFG["actboth_mod"]) == CFG["actboth_mod"] - 1
                dve_path = (not act_both) and (p % 2) < CFG["dve_mod2"]

                if p + 1 < NPAIR:
                    xb2_next = bcast(pxpool, p + 1)

                sg = spool.tile([128, CHUNK], FP16, tag="sg")
                if dve_path:
                    nc.vector.tensor_scalar(
                        out=sg[:],
                        in0=xb2[:],
                        scalar1=lowpos[:],
                        scalar2=None,
                        op0=mybir.AluOpType.is_gt,
                    )
                else:
                    nc.scalar.activation(
                        out=sg[:],
                        in_=xb2[:],
                        func=mybir.ActivationFunctionType.Sign,
                        bias=lowneg[:],
                        scale=1.0,
                    )

                ps = popool.tile([128, CHUNK], F32, tag="ps")
                vtab = v2b if dve_path else v2a
                for h in range(2):
                    nc.tensor.matmul(
                        out=ps[:, HALF * h : HALF * (h + 1)],
                        lhsT=vtab[:],
                        rhs=sg[:, HALF * h : HALF * (h + 1)],
                        start=True,
                        stop=True,
                    )

                dst = ob2[:, k * CHUNK : (k + 1) * CHUNK]
                if dve_path:
                    nc.scalar.activation(
                        out=dst,
                        in_=ps[:],
                        func=mybir.ActivationFunctionType.Identity,
                        bias=negc[:],
                        scale=1.0,
                    )
                elif act_both:
                    nc.scalar.activation(
                        out=dst,
                        in_=ps[:],
                        func=mybir.ActivationFunctionType.Copy,
                        bias=0.0,
                        scale=1.0,
                    )
                else:
                    nc.vector.tensor_copy(out=dst, in_=ps[:])

                if k == G - 1:
                    g0 = p + 1 - G
                    nc.sync.dma_start(
                        out=out_ap[:, g0 : g0 + G, :],
                        in_=ob2[0:64, :].rearrange("d (c n) -> d c n", c=G),
                    )
                    nc.gpsimd.dma_start(
                        out=out_ap[:, NPAIR + g0 : NPAIR + g0 + G, :],
                        in_=ob2[64:128, :].rearrange("d (c n) -> d c n", c=G),
                    )


_CACHED_NC = None


def _get_nc():
    global _CACHED_NC
    if _CACHED_NC is None:
        nc = bacc.Bacc("TRN2", target_bir_lowering=False, debug=False)
        x6_d = nc.dram_tensor("x6", [6, NPAIR * CHUNK], BF16, kind="ExternalInput")
        ones6_d = nc.dram_tensor("ones6", [6, 128], BF16, kind="ExternalInput")
        v2a_d = nc.dram_tensor("v2a", [128, 128], FP16, kind="ExternalInput")
        v2b_d = nc.dram_tensor("v2b", [128, 128], FP16, kind="ExternalInput")
        lowneg_d = nc.dram_tensor("lowneg", [128, 1], F32, kind="ExternalInput")
        lowpos_d = nc.dram_tensor("lowpos", [128, 1], F32, kind="ExternalInput")
        negc_d = nc.dram_tensor("negc", [128, 1], F32, kind="ExternalInput")
        out_d = nc.dram_tensor("out", [NCHUNK, D, CHUNK], FP16, kind="ExternalOutput")
        with tile.TileContext(nc) as tc:
            build_tile_kernel(
                nc, tc, x6_d, ones6_d, v2a_d, v2b_d, lowneg_d, lowpos_d, negc_d, out_d
            )
        nc.compile()
        _CACHED_NC = nc
    return _CACHED_NC


def make_host_tables(low, weight):
    """Constant device inputs derived from low/weight (float64 on host)."""
    ar = np.arange(K)
    S = 1.0 / (np.abs(ar[:, None] - ar[None, :]) + 1.0)              # [K, K] f64
    T = S @ np.asarray(weight, np.float64)                           # [K, D]
    V = np.empty_like(T)
    V[0] = (T[0] + T[-1]) / 2
    V[1:] = (T[1:] - T[:-1]) / 2

    v2a = np.zeros((128, 128), np.float16)
    v2a[0:64, 0:64] = V.astype(np.float16)
    v2a[64:128, 64:128] = v2a[0:64, 0:64]
    # DVE path: sg in {0,1}; 2V table, and -c = -sum_j V[j] folded into the
    # copy bias.  Use the fp16-rounded table when deriving c so the bias
    # cancels the table's own quantization at sg=all-ones.
    v2b16 = (2.0 * V).astype(np.float16)
    v2b = np.zeros((128, 128), np.float16)
    v2b[0:64, 0:64] = v2b16
    v2b[64:128, 64:128] = v2b16
    c = np.asarray(v2b16, np.float64).sum(axis=0) / 2                # [D] f64
    negc = np.tile((-c).astype(np.float32).reshape(1, 64), (2, 1)).reshape(128, 1)

    lowf = np.asarray(low, np.float64)
    lowneg = np.tile((-lowf).astype(np.float32), 2).reshape(128, 1)
    lowpos = np.tile(lowf.astype(np.float32), 2).reshape(128, 1)
    ones6 = np.zeros((6, 128), ml_dtypes.bfloat16)
    ones6[0:3, 0:64] = 1
    ones6[3:6, 64:128] = 1
    return {"ones6": ones6, "v2a": v2a, "v2b": v2b, "lowneg": lowneg,
            "lowpos": lowpos, "negc": negc}


def make_x6(xcore):
    """Exact hi+mid+lo bf16 split of a core's [NTOK] f32 tokens, packed as
    [6, NPAIR*CHUNK]: strands 0-2 = chunk p, strands 3-5 = chunk p+32."""
    bf = ml_dtypes.bfloat16
    x = np.ascontiguousarray(xcore, np.float32)
    hi = x.astype(bf)
    r1 = x - hi.astype(np.float32)
    mid = r1.astype(bf)
    lo = (r1 - mid.astype(np.float32)).astype(bf)
    x6 = np.empty((6, NPAIR, CHUNK), bf)
    for s, arr in enumerate((hi, mid, lo)):
        c = arr.reshape(NCHUNK, CHUNK)
        x6[s] = c[:NPAIR]
        x6[s + 3] = c[NPAIR:]
    return np.ascontiguousarray(x6.reshape(6, NPAIR * CHUNK))


def host_correct_ties(out2d, xflat, low, weight):
    """Exact fixup for tokens where x equals a bin edge: the ACT-path Sign
    gives sign(0)=0 there (averaging two table rows) while the reference uses
    strict x > low. Replace those few rows with the exact table row."""
    bins = np.asarray(low, np.float32)[1:]
    ties = np.isin(xflat, bins)
    if not ties.any():
        return out2d
    xt = xflat[ties]
    idx = (xt[:, None] > bins[None, :]).sum(-1)
    ar = np.arange(K)
    S = 1.0 / (np.abs(ar[:, None] - ar[None, :]) + 1.0)
    T = (S @ np.asarray(weight, np.float64)).astype(np.float32)
    out2d[ties] = T[idx]
    return out2d


def make_in_maps(x, low, weight):
    tabs = make_host_tables(low, weight)
    shards = np.asarray(x, np.float32).reshape(NCORES, NTOK)
    return [{"x6": make_x6(shards[i]), **tabs} for i in range(NCORES)]


def unshard(res):
    """[NCHUNK, D, CHUNK] fp16 per core -> [NCORES*NTOK, D] f32 token rows."""
    outs = []
    for i in range(NCORES):
        o = np.asarray(res.results[i]["out"])                 # [64, 64, 1024] fp16
        outs.append(o.transpose(0, 2, 1).reshape(NTOK, D).astype(np.float32))
    return np.concatenate(outs, axis=0)


def run_cores(x, low, weight, trace=False):
    nc = _get_nc()
    in_maps = make_in_maps(x, low, weight)
    res = bass_utils.run_bass_kernel_spmd(
        nc, in_maps, core_ids=list(range(NCORES)), trace=trace
    )
    return unshard(res), res


def kernel(x, low, high, weight):
    x = np.asarray(x, np.float32)
    out, _ = run_cores(x, low, weight)
    out = host_correct_ties(out, x.reshape(-1), low, weight)
    return out.reshape(B, F, D)
